# revision 56
# baseline (speedup 1.0000x reference)
"""Trainium2 Bass kernel: fused attention block (QKV proj + QK-norm + RoPE +
causal SDPA + out proj), tensor-parallel over 16 heads across 8 NeuronCores.

v2 layout strategy (evolved from the f32r baseline):
  - bf16 everywhere except PSUM accumulation and the f32 output: halves HBM
    traffic and the AllToAll payload (collective cost ~41us vs 67us each).
  - big multi-dim-AP DMAs (one per x-chunk / weight block / staging step)
    instead of ~390 small ones: each dma_start occupies its issuing queue
    ~1.6us, so DMA count is the scheduling currency.
  - V stays in SBUF (no DRAM round trip).
  - softmax denominators via transposed ones-matmuls (output moving dim 1,
    ~free on the PE) accumulated in PSUM, then reciprocal + PE transpose +
    per-128 broadcast matmuls.
  - program order interleaves batches so both AllToAlls hide under compute:
    QKV(b0) -> attn(b0) -> a2a(b0) || QKV(b1) -> attn(b1) -> a2a(b1) ||
    outproj(b0) -> outproj(b1).
  - all DMAs for x/wqkv/wo/cx ride the otherwise-idle SP (sync) queue in
    dependency order; staging + collectives + cx ride the gpsimd queue;
    output stores ride the DVE queue at the tail.
  - RoPE: d-dim host-permuted (evens to partitions 0:64, odds to 64:128);
    rotation done as qn*cosD + swap(qn)*sinD with sign-folded sinD.
  - PSUM rule: one accumulation group per bank region (start=True clears).
"""
import sys

sys.path.insert(0, "/opt/trn_rl_repo")
import numpy as np

import concourse.bacc as bacc
import concourse.mybir as mybir
from concourse.bass_utils import run_bass_kernel_spmd
from concourse.tile import TileContext

F32 = mybir.dt.float32
BF = mybir.dt.bfloat16
AF = mybir.ActivationFunctionType
MUL = mybir.AluOpType.mult

NCORES = 8
B, N, DM = 2, 2048, 2048
H, D = 16, 128
HLOC = H // NCORES          # 2 heads per core
T = B * N                   # 4096 flattened tokens
TCH = 8                     # token chunks of 512 (0-3 batch0, 4-7 batch1)
KKN = DM // 128             # 16 dm chunks
HSL = N // NCORES           # 256 tokens per core per batch after a2a

_CACHED = {}


def build():
    if "nc" in _CACHED:
        return _CACHED["nc"]
    nc = bacc.Bacc("TRN2", target_bir_lowering=False)
    _eps = nc.alloc_sbuf_tensor("const-eps", [128, 1], F32)
    nc.gpsimd.memset(_eps.ap(), 1e-6)
    nc.const_aps.aps[(F32, 1e-6)] = _eps.ap()
    nc.all_engine_barrier()

    xT = nc.dram_tensor("xT", [DM, T], BF, kind="ExternalInput")
    wqkv = nc.dram_tensor("wqkv", [DM, 6 * D], BF, kind="ExternalInput")
    wo = nc.dram_tensor("wo", [DM, DM], BF, kind="ExternalInput")
    cosd = nc.dram_tensor("cosd", [128, N], BF, kind="ExternalInput")
    sind = nc.dram_tensor("sind", [128, N], BF, kind="ExternalInput")
    masks = nc.dram_tensor("masks", [128, 4 * 512], BF, kind="ExternalInput")
    # row of 256: [0:128] = qn_g (perm'd, pre-scaled by 1/sqrt(D)), [128:256] = kn_g
    gains = nc.dram_tensor("gains", [1, 256], BF, kind="ExternalInput")
    # rows 0:256 = batch0 tokens [c*256,(c+1)*256), rows 256:512 = batch1 same
    out = nc.dram_tensor("out", [2 * HSL, DM], F32, kind="ExternalOutput")

    # multi-dim DRAM views (stride permutations)
    xT_v = xT.ap().rearrange("(kk p) t -> p kk t", p=128)          # [128,16,T]
    wqkv_v = wqkv.ap().rearrange("(kk p) f -> p kk f", p=128)      # [128,16,768]
    wo_v = wo.ap().rearrange("(kk p) f -> p kk f", p=128)          # [128,16,2048]
    out_v = out.ap().rearrange("(b t2 p) (oc f) -> b p t2 oc f", b=2, t2=2, oc=4)

    with TileContext(nc) as tc, nc.allow_low_precision(reason="bf16 compute"):
        with (
            tc.tile_pool(name="acts", bufs=1) as acts,
            tc.tile_pool(name="tmp", bufs=2) as tmp,
            tc.tile_pool(name="Pp", bufs=6) as Pp,
            tc.tile_pool(name="dram", bufs=1, space="DRAM") as dpool,
        ):
            # ---- constants: memset (no DMA); gains loaded later on SP ----
            gains_t = acts.tile([1, 256], BF, tag="gains")
            onc = acts.tile([128, 1], BF, tag="onc")
            nc.gpsimd.memset(onc[:], 1.0)
            onr = acts.tile([1, 128], BF, tag="onr")
            nc.gpsimd.memset(onr[:], 1.0)
            masks_t = acts.tile([128, 4 * 512], BF, tag="masks")
            cos_t = acts.tile([128, N], BF, tag="cos")
            sin_t = acts.tile([128, N], BF, tag="sin")

            v_sb = acts.tile([128, 32, 2 * D], BF, tag="v_sb")
            qf = [acts.tile([128, TCH, 512], BF, tag=f"qf{h}", name=f"qf{h}")
                  for h in range(HLOC)]
            kf = [acts.tile([128, TCH, 512], BF, tag=f"kf{h}", name=f"kf{h}")
                  for h in range(HLOC)]
            ctx = [acts.tile([128, TCH, 512], BF, tag=f"ctx{h}", name=f"ctx{h}")
                   for h in range(HLOC)]

            a2a_in = [[dpool.tile([1024, HSL], BF, tag=f"a2a_in{b}{h}",
                                  name=f"a2a_in{b}{h}") for h in range(HLOC)]
                      for b in range(B)]
            # one output tensor per batch; each per-head collective writes a
            # 1024-row half, so the post-collective cx load is a single DMA
            a2a_out = [dpool.tile([2048, HSL], BF, tag=f"a2a_out{b}",
                                  name=f"a2a_out{b}") for b in range(B)]
            cx_sb = [acts.tile([128, KKN, HSL], BF, tag=f"cx{b}", name=f"cx{b}")
                     for b in range(B)]

            def qkv_tch(tch, xt, psA, psN):
                pos = (tch % 4) * 512
                for oc in range(4):
                    pst = psA.tile([128, 512], F32, tag="qk")
                    ps = pst[:]
                    for kk in range(KKN):
                        nc.tensor.matmul(
                            ps, wqkv_sb[:, kk, oc * 128:(oc + 1) * 128],
                            xt[:, kk], start=(kk == 0), stop=(kk == KKN - 1))
                    if True:
                        # square + raw copy back-to-back on Act frees the
                        # PSUM tile early (the norm chain runs on the copy)
                        sq = tmp.tile([128, 512], BF, tag="sq")
                        nc.scalar.activation(sq[:], ps, AF.Square)
                        q0 = tmp.tile([128, 512], BF, tag="q0")
                        nc.scalar.copy(q0[:], ps)
                        ssum = psN.tile([1, 512], F32, tag="ssum")
                        nc.tensor.matmul(ssum[:], onc[:], sq[:], start=True, stop=True)
                        sroot = tmp.tile([1, 512], F32, tag="sroot")
                        nc.scalar.activation(sroot[:], ssum[:], AF.Sqrt,
                                             scale=1.0 / 128.0, bias=1e-6)
                        rstd = tmp.tile([1, 512], BF, tag="rstd")
                        nc.vector.reciprocal(rstd[:], sroot[:])
                        # bc[d, tok] = gain[d] * rstd[tok] (outer product on PE)
                        g = gains_t[:, 0:128] if oc < 2 else gains_t[:, 128:256]
                        bc = psN.tile([128, 512], F32, tag="bc")
                        nc.tensor.matmul(bc[:], g, rstd[:], start=True, stop=True)
                        bcs = tmp.tile([128, 512], BF, tag="bcs")
                        nc.scalar.copy(bcs[:], bc[:])
                        # rope first (norm commutes): dst = rope(q0) * bc
                        sw = tmp.tile([128, 512], BF, tag="sw")
                        nc.vector.tensor_copy(sw[0:64, :], q0[64:128, :])
                        nc.vector.tensor_copy(sw[64:128, :], q0[0:64, :])
                        nc.vector.tensor_mul(q0[:], q0[:], cos_t[:, pos:pos + 512])
                        nc.vector.tensor_mul(sw[:], sw[:], sin_t[:, pos:pos + 512])
                        nc.vector.tensor_add(q0[:], q0[:], sw[:])
                        dst = (qf[0], qf[1], kf[0], kf[1])[oc]
                        nc.vector.tensor_mul(dst[:, tch], q0[:], bcs[:])
                for tt in range(4):
                    pv = psA.tile([128, 256], F32, tag="v")
                    for kk in range(KKN):
                        nc.tensor.matmul(
                            pv[:], xt[:, kk, tt * 128:(tt + 1) * 128],
                            wqkv_sb[:, kk, 4 * D:6 * D],
                            start=(kk == 0), stop=(kk == KKN - 1))
                    nc.scalar.copy(v_sb[:, tch * 4 + tt], pv[:])

            def attn(b, psP, psC, psS, psB):
                for hh in range(HLOC):
                    for qs in range(4):
                        nkk = 4 * (qs + 1)
                        sums = psS.tile([1, 512], F32, tag="sums")
                        ctxp = psC.tile([128, 512], F32, tag="ctxp")

                        Ps = {}

                        def score_chunk(kk):
                            sps = psP.tile([128, 512], F32, tag="sps")
                            nc.tensor.matmul(
                                sps[:],
                                kf[hh][:, b * 4 + kk // 4,
                                       (kk % 4) * 128:(kk % 4 + 1) * 128],
                                qf[hh][:, b * 4 + qs], start=True, stop=True)
                            P = Pp.tile([128, 512], BF, tag="P")
                            nc.scalar.activation(P[:], sps[:], AF.Exp)
                            r = kk - 4 * qs
                            if r >= 0:  # diagonal chunk: causal mask
                                nc.vector.tensor_mul(
                                    P[:], P[:], masks_t[:, r * 512:(r + 1) * 512])
                            Ps[kk] = P

                        def accum_chunk(kk):
                            P = Ps.pop(kk)
                            nc.tensor.matmul(
                                ctxp[:], v_sb[:, b * 16 + kk, hh * 128:(hh + 1) * 128],
                                P[:], start=(kk == 0), stop=(kk == nkk - 1))
                            nc.tensor.matmul(sums[:], onc[:], P[:],
                                             start=(kk == 0), stop=(kk == nkk - 1))

                        # software pipeline: PE issues scores(k+1) before
                        # ctx/sums(k) so it never waits on the Act exp
                        score_chunk(0)
                        for kk in range(1, nkk):
                            score_chunk(kk)
                            accum_chunk(kk - 1)
                        accum_chunk(nkk - 1)
                        rrow = tmp.tile([1, 512], BF, tag="rrow")
                        nc.vector.reciprocal(rrow[:], sums[:])
                        bc2 = psB.tile([128, 512], F32, tag="bc2")
                        nc.tensor.matmul(bc2[:], onr[:], rrow[:], start=True, stop=True)
                        bc2s = tmp.tile([128, 512], BF, tag="bc2s")
                        nc.vector.tensor_copy(bc2s[:], bc2[:])
                        nc.vector.tensor_mul(ctx[hh][:, b * 4 + qs], ctxp[:], bc2s[:])
                    # -- per-head staging + AllToAll, on the gpsimd queue --
                    src = ctx[hh][:, b * 4:(b + 1) * 4, :].rearrange(
                        "p q (j2 f) -> p (q j2) f", f=HSL)
                    nc.gpsimd.dma_start(
                        a2a_in[b][hh][:].rearrange("(j p) f -> p j f", p=128), src)
                    nc.gpsimd.collective_compute(
                        "AllToAll", mybir.AluOpType.bypass,
                        replica_groups=[list(range(NCORES))],
                        ins=[a2a_in[b][hh].opt()],
                        outs=[a2a_out[b][hh * 1024:(hh + 1) * 1024, :].opt()])
                # cx loads after BOTH collectives are enqueued (a cx load
                # parked mid-queue would delay the second head's staging);
                # cx chunk order: c = hh*8+j (global feature chunk 2j+hh)
                for hh in range(HLOC):
                    nc.gpsimd.dma_start(
                        cx_sb[b][:, hh * 8:(hh + 1) * 8, :],
                        a2a_out[b][hh * 1024:(hh + 1) * 1024, :].rearrange(
                            "(j p) f -> p j f", p=128))

            def outproj(b, psO):
                # two passes: all 8 (tt,oc) blocks accumulate head-0 feature
                # chunks first (cx h0 lands one collective earlier), then
                # head-1 chunks finish each block as its cx arrives.
                psos = {}
                for tt in range(2):
                    for oc in range(4):
                        pso = psO.tile([128, 512], F32, tag=f"pso{tt}{oc}",
                                       name=f"pso{tt}{oc}")
                        psos[(tt, oc)] = pso
                        for j in range(8):
                            nc.tensor.matmul(
                                pso[:], cx_sb[b][:, j, tt * 128:(tt + 1) * 128],
                                wo_sb[:, oc, 2 * j], start=(j == 0), stop=False,
                                skip_group_check=True)
                for tt in range(2):
                    ot = otp.tile([128, 4, 512], F32, tag="ot")
                    for oc in range(4):
                        pso = psos[(tt, oc)]
                        for j in range(8):
                            nc.tensor.matmul(
                                pso[:], cx_sb[b][:, 8 + j, tt * 128:(tt + 1) * 128],
                                wo_sb[:, oc, 2 * j + 1], start=False, stop=(j == 7),
                                skip_group_check=True)
                        nc.scalar.copy(ot[:, oc], pso[:])
                        if oc % 2 == 1:  # store oc-pairs: shorter drain tail
                            nc.sync.dma_start(
                                out_v[b][:, tt, oc - 1:oc + 1], ot[:, oc - 1:oc + 1])

            # ================= phase 1: QKV both batches + attn b0 ==========
            with tc.tile_pool(name="xw", bufs=1) as xw:
                # wqkv on the Act queue, x on SP, both split so the first
                # kk-chunks land early and the first matmuls start ~7us in.
                wqkv_sb = xw.tile([128, KKN, 6 * D], BF, tag="wqkv")
                nc.scalar.dma_start(wqkv_sb[:, 0:4], wqkv_v[:, 0:4])
                nc.scalar.dma_start(wqkv_sb[:, 4:16], wqkv_v[:, 4:16])
                # rope/mask tables after the weights in Act-queue order
                # (needed only ~20us in; keeps the DMA device clear early)
                nc.scalar.dma_start(cos_t[:], cosd[:])
                nc.scalar.dma_start(sin_t[:], sind[:])
                nc.scalar.dma_start(masks_t[:], masks[:])
                with tc.tile_pool(name="xp", bufs=2) as xp:
                    xts = []
                    for tch in range(TCH):
                        xt = xp.tile([128, KKN, 512], BF, tag="x")
                        if tch == 0:
                            nc.sync.dma_start(
                                xt[:, 0:4], xT_v[:, 0:4, 0:512])
                            nc.sync.dma_start(
                                xt[:, 4:16], xT_v[:, 4:16, 0:512])
                            nc.sync.dma_start(gains_t[:], gains[:])
                        else:
                            nc.sync.dma_start(
                                xt[:], xT_v[:, :, tch * 512:(tch + 1) * 512])
                        xts.append(xt)

                    with (
                        tc.tile_pool(name="psA0", bufs=3, space="PSUM") as psA,
                        tc.tile_pool(name="psN0", bufs=1, space="PSUM") as psN,
                    ):
                        for tch in range(4):
                            qkv_tch(tch, xts[tch], psA, psN)
                    with (
                        tc.tile_pool(name="psP0", bufs=3, space="PSUM") as psP,
                        tc.tile_pool(name="psC0", bufs=2, space="PSUM") as psC,
                        tc.tile_pool(name="psS0", bufs=2, space="PSUM") as psS,
                        tc.tile_pool(name="psB0", bufs=1, space="PSUM") as psB,
                    ):
                        attn(0, psP, psC, psS, psB)
                    with (
                        tc.tile_pool(name="psA1", bufs=3, space="PSUM") as psA,
                        tc.tile_pool(name="psN1", bufs=1, space="PSUM") as psN,
                    ):
                        for tch in range(4, 8):
                            qkv_tch(tch, xts[tch], psA, psN)

            # ================= phase 2: attn b1 + out-proj ==================
            with tc.tile_pool(name="wop", bufs=1) as wop:
                wo_sb = wop.tile([128, 4, KKN, 512], BF, tag="wo")
                for oc in range(4):
                    nc.sync.dma_start(
                        wo_sb[:, oc], wo_v[:, :, oc * 512:(oc + 1) * 512])
                with (
                    tc.tile_pool(name="psP1", bufs=3, space="PSUM") as psP,
                    tc.tile_pool(name="psC1", bufs=2, space="PSUM") as psC,
                    tc.tile_pool(name="psS1", bufs=2, space="PSUM") as psS,
                    tc.tile_pool(name="psB1", bufs=1, space="PSUM") as psB,
                ):
                    attn(1, psP, psC, psS, psB)
                with (
                    tc.tile_pool(name="psO", bufs=1, space="PSUM") as psO,
                    tc.tile_pool(name="otp", bufs=3) as otp,
                ):
                    outproj(0, psO)
                    outproj(1, psO)

    nc.compile()
    _CACHED["nc"] = nc
    return nc


def _host_inputs(x, w_qkv, w_out, qn_g, kn_g):
    import ml_dtypes
    bf = ml_dtypes.bfloat16
    x = np.asarray(x, dtype=np.float32)
    w_qkv = np.asarray(w_qkv, dtype=np.float32)
    w_out = np.asarray(w_out, dtype=np.float32)
    qn_g = np.asarray(qn_g, dtype=np.float32)
    kn_g = np.asarray(kn_g, dtype=np.float32)

    perm = np.concatenate([np.arange(0, D, 2), np.arange(1, D, 2)])
    xT = np.ascontiguousarray(x.reshape(T, DM).T).astype(bf)
    woT = np.ascontiguousarray(w_out.T).astype(bf)

    # rope tables (position within a batch), duplicated halves / sign-folded
    inv = 1.0 / (10000.0 ** (np.arange(0, D, 2, dtype=np.float64) / D))  # [64]
    ang = np.arange(N, dtype=np.float64)[:, None] * inv[None, :]         # [N, 64]
    c0 = np.cos(ang).T.astype(np.float32)                                # [64, N]
    s0 = np.sin(ang).T.astype(np.float32)
    cosd = np.vstack([c0, c0]).astype(bf)
    sind = np.vstack([-s0, s0]).astype(bf)

    # causal diagonal masks, 4 variants r=0..3: allowed iff 128*r + p <= j
    p = np.arange(128)[:, None]
    j = np.arange(512)[None, :]
    masks = np.concatenate(
        [(128 * r + p <= j).astype(np.float32) for r in range(4)], axis=1).astype(bf)

    gains = np.concatenate([qn_g[perm] / np.sqrt(np.float32(D)),
                            kn_g[perm]]).reshape(1, 256).astype(bf)

    shared = {
        "xT": xT, "wo": woT, "cosd": cosd, "sind": sind, "masks": masks,
        "gains": gains,
    }
    in_maps = []
    for c in range(NCORES):
        hs = [HLOC * c + i for i in range(HLOC)]
        q_rows = np.concatenate([(0 * H + h) * D + perm for h in hs])
        k_rows = np.concatenate([(1 * H + h) * D + perm for h in hs])
        v_rows = np.concatenate([(2 * H + h) * D + np.arange(D) for h in hs])
        rows = np.concatenate([q_rows, k_rows, v_rows])
        wqkv_c = np.ascontiguousarray(w_qkv[rows, :].T).astype(bf)
        in_maps.append({**shared, "wqkv": wqkv_c})
    return in_maps


def kernel(x, w_qkv, w_out, qn_g, kn_g):
    nc = build()
    in_maps = _host_inputs(x, w_qkv, w_out, qn_g, kn_g)
    res = run_bass_kernel_spmd(nc, in_maps, list(range(NCORES)))
    out = np.empty((B, N, DM), dtype=np.float32)
    for c in range(NCORES):
        o = res.results[c]["out"]
        for b in range(B):
            out[b, c * HSL:(c + 1) * HSL, :] = o[b * HSL:(b + 1) * HSL, :]
    return out
